# revision 4
# baseline (speedup 1.0000x reference)
"""Graphormer attention head on 8 trn2 NeuronCores (row-parallel).

out = softmax(mask(q@k.T/8, adj)) @ v  with q/k/v = x@W+b, adj scattered
from edge_index.

Sharding: core c owns output rows [c*1024, (c+1)*1024). Per core:
- x^T streamed as bf16 (half the HBM traffic of f32, and bf16 moving
  operands run the PE at 1 cycle/row vs 4 for f32).
- K and V projected in ONE joint pass: W_kv = [Wk | Wv] (256x128), so
  kv^T = W_kv^T @ x^T lands as a [128, 8192] fp16 tile (rows 0-63 = k^T,
  rows 64-127 = v^T). v^T is transposed to row-major [128, 64] tiles via
  the DMA XBAR (dma_start_transpose), costing no compute-engine cycles.
- Scores: single fp16 matmul per 512-col half (q/k fp16 rounding gives
  ~7e-4 score error, far inside the 2e-2 output tolerance).
- Mask is host-built as {0,1} fp16 (all-16-bit operands let the DVE
  multiply run in 2x mode); w = exp(s) * m, and the softmax denominator
  comes free via a ones-column appended to V.
- Final: transpose acc via identity matmul, divide by Z.
"""
import os
import sys

for _p in ("/opt/trn_rl_repo", "/root/.axon_site/_ro/trn_rl_repo"):
    if os.path.isdir(_p) and _p not in sys.path:
        sys.path.insert(0, _p)

import numpy as np
import ml_dtypes

import concourse.bass as bass
import concourse.bacc as bacc
import concourse.mybir as mybir
import concourse.tile as tile
from concourse.bass_utils import run_bass_kernel_spmd

N = 8192
DIN = 256
DQ = 64
NCORES = 8
NLOC = N // NCORES          # 1024 rows per core
JT = N // 128               # 64 column tiles of 128
SEG = 512
VSLOT = 80                  # v-tile slot width; 65 would misalign the DMA-XBAR writes
F32 = mybir.dt.float32
F16 = mybir.dt.float16
BF16 = mybir.dt.bfloat16


def _emit(nc, tc, ctx):
    from concourse.mybir import AluOpType as AO, ActivationFunctionType as AF

    xt = nc.dram_tensor("xt", [DIN, N], BF16, kind="ExternalInput")
    xtq = nc.dram_tensor("xtq", [DIN, NLOC], BF16, kind="ExternalInput")
    wq = nc.dram_tensor("wq", [DIN, DQ], BF16, kind="ExternalInput")
    wkv = nc.dram_tensor("wkv", [DIN, 128], BF16, kind="ExternalInput")
    bq = nc.dram_tensor("bq", [DQ, 1], F32, kind="ExternalInput")
    bkv = nc.dram_tensor("bkv", [128, 1], F32, kind="ExternalInput")
    i65 = nc.dram_tensor("i65", [DQ + 1, DQ + 1], F32, kind="ExternalInput")
    maskt = nc.dram_tensor("maskt", [N, NLOC], F16, kind="ExternalInput")
    out = nc.dram_tensor("out", [NLOC, DQ], F32, kind="ExternalOutput")

    pers = ctx.enter_context(tc.tile_pool(name="pers", bufs=1))
    pm = ctx.enter_context(tc.tile_pool(name="pm", bufs=5))
    pe_ = ctx.enter_context(tc.tile_pool(name="pe", bufs=3))
    pw = ctx.enter_context(tc.tile_pool(name="pw", bufs=3))
    pfin = ctx.enter_context(tc.tile_pool(name="pfin", bufs=2))
    ps = ctx.enter_context(tc.tile_pool(name="ps", bufs=3, space="PSUM"))
    pacc = ctx.enter_context(tc.tile_pool(name="pacc", bufs=1, space="PSUM"))

    # ---- persistent SBUF ----
    xt_sb = [pers.tile([128, N], BF16, tag=f"xt{c}", name=f"xt{c}") for c in range(2)]
    xtq_sb = [pers.tile([128, NLOC], BF16, tag=f"xtq{c}", name=f"xtq{c}") for c in range(2)]
    wq_sb = [pers.tile([128, DQ], BF16, tag=f"wq{c}", name=f"wq{c}")
             for c in range(2)]
    wkv_sb = [pers.tile([128, 128], BF16, tag=f"wkv{c}", name=f"wkv{c}")
              for c in range(2)]
    for c in range(2):
        nc.sync.dma_start(wq_sb[c][:], wq[c * 128:(c + 1) * 128, :])
        nc.sync.dma_start(wkv_sb[c][:], wkv[c * 128:(c + 1) * 128, :])
    bq_sb = pers.tile([DQ, 1], F32, tag="bq")
    bkv_sb = pers.tile([128, 1], F32, tag="bkv")
    i65_sb = pers.tile([DQ + 1, DQ + 1], F32, tag="i65")
    nc.sync.dma_start(bq_sb[:], bq[:])
    nc.sync.dma_start(bkv_sb[:], bkv[:])
    nc.sync.dma_start(i65_sb[:], i65[:])
    qth_sb = pers.tile([DQ, NLOC], F16, tag="qth")
    kvt_sb = pers.tile([128, N], F16, tag="kvt")
    vh_sb = pers.tile([128, JT * VSLOT], F16, tag="vh")
    accT_sb = pers.tile([DQ + 1, NLOC], F32, tag="accT")

    # x^T streamed in 512-col segments so projections can start early
    for c in range(2):
        nc.sync.dma_start(xtq_sb[c][:], xtq[c * 128:(c + 1) * 128, :])
        for s in range(N // SEG):
            nc.sync.dma_start(
                xt_sb[c][:, s * SEG:(s + 1) * SEG],
                xt[c * 128:(c + 1) * 128, s * SEG:(s + 1) * SEG],
            )

    vh3 = vh_sb[:].rearrange("p (b e) -> p b e", e=VSLOT)
    nc.vector.memset(vh3[:, :, DQ:DQ + 1], 1.0)

    # ---- Q projection: q^T [64, 1024] fp16 (scale/bias folded on host) ----
    qt = ps.tile([128, NLOC], F32, tag="s", name="qt")
    for h in range(2):
        hs = slice(h * SEG, (h + 1) * SEG)
        nc.tensor.matmul(qt[:DQ, hs], wq_sb[0][:], xtq_sb[0][:, hs],
                         start=True, stop=False)
        nc.tensor.matmul(qt[:DQ, hs], wq_sb[1][:], xtq_sb[1][:, hs],
                         start=False, stop=True)
    nc.vector.tensor_scalar_add(qth_sb[:], qt[:DQ, :], bq_sb[:])

    # ---- joint K/V projection: kv^T [128, 8192] fp16, then v tiles via
    # DMA-XBAR transpose into row-major [128, 64] blocks of vh3 ----
    for c16 in range(N // SEG):
        cs = slice(c16 * SEG, (c16 + 1) * SEG)
        t = ps.tile([128, NLOC], F32, tag="s", name=f"kv{c16}")
        nc.tensor.matmul(t[:, 0:SEG], wkv_sb[0][:], xt_sb[0][:, cs],
                         start=True, stop=False)
        nc.tensor.matmul(t[:, 0:SEG], wkv_sb[1][:], xt_sb[1][:, cs],
                         start=False, stop=True)
        nc.vector.tensor_scalar_add(kvt_sb[:, cs], t[:, 0:SEG], bkv_sb[:])
        for b in range(SEG // 128):
            jt = c16 * (SEG // 128) + b
            nc.sync.dma_start_transpose(
                vh3[:, jt, 0:DQ],
                kvt_sb[DQ:128, jt * 128:(jt + 1) * 128],
            )

    # ---- main loop over 64 column tiles ----
    acc = pacc.tile([DQ + 1, NLOC], F32, tag="acc")
    for jt in range(JT):
        m_t = pm.tile([128, NLOC], F16, tag="m")
        nc.sync.dma_start(m_t[:], maskt[jt * 128:(jt + 1) * 128, :])
        s_t = ps.tile([128, NLOC], F32, tag="s")
        kh = kvt_sb[0:DQ, jt * 128:(jt + 1) * 128]
        for h in range(2):
            hs = slice(h * SEG, (h + 1) * SEG)
            nc.tensor.matmul(s_t[:, hs], kh, qth_sb[:, hs],
                             start=True, stop=True)
        e_t = pe_.tile([128, NLOC], F16, tag="e")
        nc.scalar.activation(e_t[:], s_t[:], AF.Exp)
        w_t = pw.tile([128, NLOC], F16, tag="w")
        nc.vector.tensor_tensor(w_t[:], e_t[:], m_t[:], AO.mult)
        vhb = vh3[:, jt, 0:DQ + 1]
        for h in range(2):
            hs = slice(h * SEG, (h + 1) * SEG)
            nc.tensor.matmul(acc[:, hs], vhb, w_t[:, hs],
                             start=(jt == 0), stop=(jt == JT - 1))

    # ---- finish: transpose via matmul with I65, divide by Z ----
    nc.scalar.activation(accT_sb[:], acc[:], AF.Copy)
    for it in range(NLOC // 128):
        po = ps.tile([128, NLOC], F32, tag="s", name=f"po{it}")
        nc.tensor.matmul(po[:, 0:DQ + 1], accT_sb[:, it * 128:(it + 1) * 128],
                         i65_sb[:], start=True, stop=True)
        rz = pfin.tile([128, 1], F32, tag="rz")
        nc.vector.reciprocal(rz[:], po[:, DQ:DQ + 1])
        o_t = pfin.tile([128, DQ], F32, tag="o")
        nc.vector.tensor_scalar_mul(o_t[:], po[:, 0:DQ], rz[:])
        nc.sync.dma_start(out[it * 128:(it + 1) * 128, :], o_t[:])


_CACHE = {}


def _program():
    if "nc" not in _CACHE:
        import contextlib
        nc = bacc.Bacc("TRN2", target_bir_lowering=False, debug=False,
                       num_devices=NCORES)
        with tile.TileContext(nc) as tc:
            with contextlib.ExitStack() as ctx:
                _emit(nc, tc, ctx)
        nc.compile()
        _CACHE["nc"] = nc
    return _CACHE["nc"]


def kernel(**inputs):
    x = np.asarray(inputs["x"], dtype=np.float32)
    ei = np.asarray(inputs["edge_index"])
    Wq = np.asarray(inputs["Wq"], dtype=np.float32)
    bq = np.asarray(inputs["bq"], dtype=np.float32)
    Wk = np.asarray(inputs["Wk"], dtype=np.float32)
    bk = np.asarray(inputs["bk"], dtype=np.float32)
    Wv = np.asarray(inputs["Wv"], dtype=np.float32)
    bv = np.asarray(inputs["bv"], dtype=np.float32)

    scale = 1.0 / np.sqrt(np.float32(DQ))
    bf16 = ml_dtypes.bfloat16
    xT = np.ascontiguousarray(x.T).astype(bf16)         # (256, 8192)
    wq_s = np.ascontiguousarray(Wq * scale).astype(bf16)
    wkv = np.ascontiguousarray(np.concatenate([Wk, Wv], axis=1)).astype(bf16)
    bq_s = np.ascontiguousarray((bq * scale).reshape(DQ, 1))
    bkv = np.ascontiguousarray(np.concatenate([bk, bv]).reshape(128, 1))
    i65 = np.eye(DQ + 1, dtype=np.float32)
    adj = np.zeros((N, N), dtype=np.bool_)
    adj[ei[0], ei[1]] = True

    in_maps = []
    for c in range(NCORES):
        rows = slice(c * NLOC, (c + 1) * NLOC)
        in_maps.append({
            "xt": xT,
            "xtq": np.ascontiguousarray(xT[:, rows]),
            "wq": wq_s, "wkv": wkv,
            "bq": bq_s, "bkv": bkv, "i65": i65,
            "maskt": np.ascontiguousarray(adj[rows].T).astype(np.float16),
        })

    global _last_in_maps
    _last_in_maps = in_maps
    nc = _program()
    res = run_bass_kernel_spmd(nc, in_maps, core_ids=list(range(NCORES)))
    out = np.concatenate([res.results[c]["out"] for c in range(NCORES)], axis=0)
    return out.astype(np.float32)


_last_in_maps = None


# revision 8
# speedup vs baseline: 1.4688x; 1.4688x over previous
"""Graphormer attention head on 8 trn2 NeuronCores (row-parallel).

out = softmax(mask(q@k.T/8, adj)) @ v  with q/k/v = x@W+b, adj scattered
from edge_index.

Sharding: core c owns output rows [c*1024, (c+1)*1024). Per core:
- x^T streamed as bf16 (half the HBM traffic of f32; bf16 moving
  operands run the PE at 1 cycle/row vs 4 for f32).
- K and V projected in ONE joint pass: W_kv = [Wk | Wv] (256x128), so
  kv^T lands as a [128, 8192] fp16 tile (rows 0-63 = k^T, rows 64-127 =
  v^T). v^T is transposed to row-major [128, 64] tiles via the DMA XBAR
  (dma_start_transpose) -- no compute-engine cycles.
- Scores: single fp16 matmul per 512-col half (~7e-4 score error, far
  inside the 2e-2 output tolerance).
- Mask: fp8 {0, -30} tiles accumulated straight into the score PSUM by
  an fp8 DoubleRow identity matmul (256 PE cycles per half). exp(s-30)
  flushes to exactly 0 in fp16, so the exp output IS the masked weight
  tile -- there is no separate mask multiply and the vector engine stays
  off the critical path. The softmax denominator comes free via a
  ones-column appended to V.
- Final: transpose acc via identity matmul, divide by Z, one output DMA.
"""
import os
import sys

for _p in ("/opt/trn_rl_repo", "/root/.axon_site/_ro/trn_rl_repo"):
    if os.path.isdir(_p) and _p not in sys.path:
        sys.path.insert(0, _p)

import numpy as np
import ml_dtypes

import concourse.bass as bass
import concourse.bacc as bacc
import concourse.mybir as mybir
import concourse.tile as tile
from concourse.bass_utils import run_bass_kernel_spmd

N = 8192
DIN = 256
DQ = 64
NCORES = 8
NLOC = N // NCORES          # 1024 rows per core
JT = N // 128               # 64 column tiles of 128
SEG = 512
XSEG = 2048                 # x^T DMA segment width
VSLOT = 80                  # v-tile slot width; 65 would misalign the DMA-XBAR
MNEG = -30.0                # mask bias; exp(s-30) underflows fp16 to exact 0
F32 = mybir.dt.float32
F16 = mybir.dt.float16
BF16 = mybir.dt.bfloat16
FP8 = mybir.dt.float8e4


def _emit(nc, tc, ctx):
    from concourse.mybir import AluOpType as AO, ActivationFunctionType as AF
    DR = mybir.MatmulPerfMode.DoubleRow

    xt = nc.dram_tensor("xt", [DIN, N], BF16, kind="ExternalInput")
    xtq = nc.dram_tensor("xtq", [DIN, NLOC], BF16, kind="ExternalInput")
    wq = nc.dram_tensor("wq", [DIN, DQ], BF16, kind="ExternalInput")
    wkv = nc.dram_tensor("wkv", [DIN, 128], BF16, kind="ExternalInput")
    wsel = nc.dram_tensor("wsel", [2, 128, 2 * 128], FP8, kind="ExternalInput")
    bq = nc.dram_tensor("bq", [DQ, 1], F32, kind="ExternalInput")
    bkv = nc.dram_tensor("bkv", [128, 1], F32, kind="ExternalInput")
    i65 = nc.dram_tensor("i65", [DQ + 1, DQ + 1], F16, kind="ExternalInput")
    maskt = nc.dram_tensor("maskt", [N, NLOC], FP8, kind="ExternalInput")
    out = nc.dram_tensor("out", [NLOC, DQ], F32, kind="ExternalOutput")

    pers = ctx.enter_context(tc.tile_pool(name="pers", bufs=1))
    pm = ctx.enter_context(tc.tile_pool(name="pm", bufs=6))
    pw = ctx.enter_context(tc.tile_pool(name="pw", bufs=3))
    pfin = ctx.enter_context(tc.tile_pool(name="pfin", bufs=2))
    ps = ctx.enter_context(tc.tile_pool(name="ps", bufs=3, space="PSUM"))
    pacc = ctx.enter_context(tc.tile_pool(name="pacc", bufs=1, space="PSUM"))

    # ---- persistent SBUF ----
    xt_sb = [pers.tile([128, N], BF16, tag=f"xt{c}", name=f"xt{c}") for c in range(2)]
    xtq_sb = [pers.tile([128, NLOC], BF16, tag=f"xtq{c}", name=f"xtq{c}") for c in range(2)]
    wq_sb = [pers.tile([128, DQ], BF16, tag=f"wq{c}", name=f"wq{c}")
             for c in range(2)]
    wkv_sb = [pers.tile([128, 128], BF16, tag=f"wkv{c}", name=f"wkv{c}")
              for c in range(2)]
    wsel_sb = [pers.tile([128, 2 * 128], FP8, tag=f"wsel{h}", name=f"wsel{h}")
               for h in range(2)]
    for c in range(2):
        nc.sync.dma_start(wq_sb[c][:], wq[c * 128:(c + 1) * 128, :])
        nc.sync.dma_start(wkv_sb[c][:], wkv[c * 128:(c + 1) * 128, :])
        nc.sync.dma_start(wsel_sb[c][:], wsel[c, :, :])
    bq_sb = pers.tile([DQ, 1], F32, tag="bq")
    bkv_sb = pers.tile([128, 1], F32, tag="bkv")
    i65_sb = pers.tile([DQ + 1, DQ + 1], F16, tag="i65")
    nc.sync.dma_start(bq_sb[:], bq[:])
    nc.sync.dma_start(bkv_sb[:], bkv[:])
    nc.sync.dma_start(i65_sb[:], i65[:])
    qth_sb = pers.tile([DQ, NLOC], F16, tag="qth")
    kvt_sb = pers.tile([128, N], F16, tag="kvt")
    vh_sb = pers.tile([128, JT * VSLOT], F16, tag="vh")
    accT_sb = pers.tile([DQ + 1, NLOC], F16, tag="accT")
    o_sb = pers.tile([128, (NLOC // 128) * DQ], F32, tag="osb")

    # x^T streamed in segments, seg-major so both halves of segment 0 land
    # first and the projections can start early
    for c in range(2):
        nc.sync.dma_start(xtq_sb[c][:], xtq[c * 128:(c + 1) * 128, :])
    for s in range(N // XSEG):
        for c in range(2):
            nc.sync.dma_start(
                xt_sb[c][:, s * XSEG:(s + 1) * XSEG],
                xt[c * 128:(c + 1) * 128, s * XSEG:(s + 1) * XSEG],
            )

    vh3 = vh_sb[:].rearrange("p (b e) -> p b e", e=VSLOT)
    nc.vector.memset(vh3[:, :, DQ:DQ + 1], 1.0)
    sel3 = [wsel_sb[h][:].rearrange("p (a b) -> p a b", b=128) for h in range(2)]

    # ---- Q projection: q^T [64, 1024] fp16 (scale/bias folded on host) ----
    qt = ps.tile([128, NLOC], F32, tag="s", name="qt")
    for h in range(2):
        hs = slice(h * SEG, (h + 1) * SEG)
        nc.tensor.matmul(qt[:DQ, hs], wq_sb[0][:], xtq_sb[0][:, hs],
                         start=True, stop=False)
        nc.tensor.matmul(qt[:DQ, hs], wq_sb[1][:], xtq_sb[1][:, hs],
                         start=False, stop=True)
    nc.vector.tensor_scalar_add(qth_sb[:], qt[:DQ, :], bq_sb[:])

    # ---- joint K/V projection: kv^T [128, 8192] fp16; v tiles row-major
    # via batched DMA-XBAR transposes (16 tiles per call). Chunks 0-3 run
    # in the preamble; 4-15 interleave into the main loop so the PE stays
    # continuously busy (and in its fast p-state) instead of stalling on
    # the DVE drain round-trip. ----
    def kv_chunk(c16):
        cs = slice(c16 * SEG, (c16 + 1) * SEG)
        t = ps.tile([128, NLOC], F32, tag="s", name=f"kv{c16}")
        nc.tensor.matmul(t[:, 0:SEG], wkv_sb[0][:], xt_sb[0][:, cs],
                         start=True, stop=False)
        nc.tensor.matmul(t[:, 0:SEG], wkv_sb[1][:], xt_sb[1][:, cs],
                         start=False, stop=True)
        nc.vector.tensor_scalar_add(kvt_sb[:, cs], t[:, 0:SEG], bkv_sb[:])

    def v_transpose(c16_lo, c16_hi):
        jt0, jt1 = c16_lo * (SEG // 128), c16_hi * (SEG // 128)
        nc.sync.dma_start_transpose(
            vh3[:, jt0:jt1, 0:DQ],
            kvt_sb[DQ:128, c16_lo * SEG:c16_hi * SEG],
        )

    for c16 in range(4):
        kv_chunk(c16)
    v_transpose(0, 4)

    # ---- main loop over 64 column tiles ----
    acc = pacc.tile([DQ + 1, NLOC], F32, tag="acc")
    for jt in range(JT):
        m_t = pm.tile([128, NLOC], FP8, tag="m")
        nc.sync.dma_start(m_t[:], maskt[jt * 128:(jt + 1) * 128, :])
        m3 = m_t[:].rearrange("p (a b) -> p a b", b=SEG)
        s_t = ps.tile([128, NLOC], F32, tag="s")
        kh = kvt_sb[0:DQ, jt * 128:(jt + 1) * 128]
        for h in range(2):
            hs = slice(h * SEG, (h + 1) * SEG)
            nc.tensor.matmul(s_t[:, hs], kh, qth_sb[:, hs],
                             start=True, stop=False)
        for h in range(2):
            hs = slice(h * SEG, (h + 1) * SEG)
            nc.tensor.matmul(s_t[:, hs], sel3[h], m3, start=False, stop=True,
                             perf_mode=DR, skip_group_check=True)
        w_t = pw.tile([128, NLOC], F16, tag="w")
        nc.scalar.activation(w_t[:], s_t[:], AF.Exp)
        vhb = vh3[:, jt, 0:DQ + 1]
        for h in range(2):
            hs = slice(h * SEG, (h + 1) * SEG)
            nc.tensor.matmul(acc[:, hs], vhb, w_t[:, hs],
                             start=(jt == 0), stop=(jt == JT - 1))
        if jt < 12:
            kv_chunk(4 + jt)
            if jt % 4 == 3:
                v_transpose(jt + 1, jt + 5)

    # ---- finish: transpose via matmul with I65, divide by Z, one DMA ----
    nc.scalar.activation(accT_sb[:], acc[:], AF.Copy)
    o3 = o_sb[:].rearrange("p (b e) -> p b e", e=DQ)
    for it in range(NLOC // 128):
        po = ps.tile([128, NLOC], F32, tag="s", name=f"po{it}")
        nc.tensor.matmul(po[:, 0:DQ + 1], accT_sb[:, it * 128:(it + 1) * 128],
                         i65_sb[:], start=True, stop=True)
        rz = pfin.tile([128, 1], F32, tag="rz")
        nc.vector.reciprocal(rz[:], po[:, DQ:DQ + 1])
        nc.vector.tensor_scalar_mul(o3[:, it, :], po[:, 0:DQ], rz[:])
    out3 = out[:].rearrange("(b p) e -> p b e", p=128)
    nc.scalar.dma_start(out3, o3[:, :, :])


_CACHE = {}


def _program():
    if "nc" not in _CACHE:
        import contextlib
        nc = bacc.Bacc("TRN2", target_bir_lowering=False, debug=False,
                       num_devices=NCORES)
        with tile.TileContext(nc) as tc:
            with contextlib.ExitStack() as ctx:
                _emit(nc, tc, ctx)
        nc.compile()
        _CACHE["nc"] = nc
    return _CACHE["nc"]


def kernel(**inputs):
    x = np.asarray(inputs["x"], dtype=np.float32)
    ei = np.asarray(inputs["edge_index"])
    Wq = np.asarray(inputs["Wq"], dtype=np.float32)
    bq = np.asarray(inputs["bq"], dtype=np.float32)
    Wk = np.asarray(inputs["Wk"], dtype=np.float32)
    bk = np.asarray(inputs["bk"], dtype=np.float32)
    Wv = np.asarray(inputs["Wv"], dtype=np.float32)
    bv = np.asarray(inputs["bv"], dtype=np.float32)

    scale = 1.0 / np.sqrt(np.float32(DQ))
    bf16 = ml_dtypes.bfloat16
    f8 = ml_dtypes.float8_e4m3
    xT = np.ascontiguousarray(x.T).astype(bf16)         # (256, 8192)
    wq_s = np.ascontiguousarray(Wq * scale).astype(bf16)
    wkv = np.ascontiguousarray(np.concatenate([Wk, Wv], axis=1)).astype(bf16)
    bq_s = np.ascontiguousarray((bq * scale).reshape(DQ, 1))
    bkv = np.ascontiguousarray(np.concatenate([bk, bv]).reshape(128, 1))
    i65 = np.eye(DQ + 1, dtype=np.float16)
    # DoubleRow mask-add selectors: wsel[h][q, sub, p] = (sub == h and q == p)
    wsel = np.zeros((2, 128, 2, 128), dtype=np.float32)
    for h in range(2):
        wsel[h, np.arange(128), h, np.arange(128)] = 1.0
    wsel = wsel.reshape(2, 128, 2 * 128).astype(f8)
    adj = np.zeros((N, N), dtype=np.bool_)
    adj[ei[0], ei[1]] = True

    in_maps = []
    for c in range(NCORES):
        rows = slice(c * NLOC, (c + 1) * NLOC)
        mt = np.where(adj[rows].T, np.float32(0.0), np.float32(MNEG))
        in_maps.append({
            "xt": xT,
            "xtq": np.ascontiguousarray(xT[:, rows]),
            "wq": wq_s, "wkv": wkv, "wsel": wsel,
            "bq": bq_s, "bkv": bkv, "i65": i65,
            "maskt": np.ascontiguousarray(mt).astype(f8),
        })

    global _last_in_maps
    _last_in_maps = in_maps
    nc = _program()
    res = run_bass_kernel_spmd(nc, in_maps, core_ids=list(range(NCORES)))
    out = np.concatenate([res.results[c]["out"] for c in range(NCORES)], axis=0)
    return out.astype(np.float32)


_last_in_maps = None


# revision 9
# speedup vs baseline: 1.6444x; 1.1196x over previous
"""Graphormer attention head on 8 trn2 NeuronCores (row-parallel).

out = softmax(mask(q@k.T/8, adj)) @ v  with q/k/v = x@W+b, adj scattered
from edge_index.

Sharding: core c owns output rows [c*1024, (c+1)*1024). Per core:
- x^T streamed as bf16 (half the HBM traffic of f32; bf16 moving
  operands run the PE at 1 cycle/row vs 4 for f32).
- K and V projected in ONE joint pass: W_kv = [Wk | Wv] (256x128), so
  kv^T lands as a [128, 8192] fp16 tile (rows 0-63 = k^T, rows 64-127 =
  v^T). v^T is transposed to row-major [128, 64] tiles via the DMA XBAR
  (dma_start_transpose) -- no compute-engine cycles.
- Scores: single fp16 matmul per 512-col half (~7e-4 score error, far
  inside the 2e-2 output tolerance).
- Mask: fp8 {0, -30} tiles accumulated straight into the score PSUM by
  an fp8 DoubleRow identity matmul (256 PE cycles per half). exp(s-30)
  flushes to exactly 0 in fp16, so the exp output IS the masked weight
  tile -- there is no separate mask multiply and the vector engine stays
  off the critical path. The softmax denominator comes free via a
  ones-column appended to V.
- Final: transpose acc via identity matmul, divide by Z, one output DMA.
"""
import os
import sys

for _p in ("/opt/trn_rl_repo", "/root/.axon_site/_ro/trn_rl_repo"):
    if os.path.isdir(_p) and _p not in sys.path:
        sys.path.insert(0, _p)

import numpy as np
import ml_dtypes

import concourse.bass as bass
import concourse.bacc as bacc
import concourse.mybir as mybir
import concourse.tile as tile
from concourse.bass_utils import run_bass_kernel_spmd

N = 8192
DIN = 256
DQ = 64
NCORES = 8
NLOC = N // NCORES          # 1024 rows per core
JT = N // 128               # 64 column tiles of 128
SEG = 512
XSEG = 2048                 # x^T DMA segment width
VSLOT = 80                  # v-tile slot width; 65 would misalign the DMA-XBAR
MNEG = -30.0                # mask bias; exp(s-30) underflows fp16 to exact 0
F32 = mybir.dt.float32
F16 = mybir.dt.float16
BF16 = mybir.dt.bfloat16
FP8 = mybir.dt.float8e4


def _emit(nc, tc, ctx):
    from concourse.mybir import AluOpType as AO, ActivationFunctionType as AF
    DR = mybir.MatmulPerfMode.DoubleRow

    xt = nc.dram_tensor("xt", [DIN, N], BF16, kind="ExternalInput")
    xtq = nc.dram_tensor("xtq", [DIN, NLOC], BF16, kind="ExternalInput")
    wq = nc.dram_tensor("wq", [DIN, DQ], BF16, kind="ExternalInput")
    wkv = nc.dram_tensor("wkv", [DIN, 128], BF16, kind="ExternalInput")
    bq = nc.dram_tensor("bq", [DQ, 1], F32, kind="ExternalInput")
    bkv = nc.dram_tensor("bkv", [128, 1], F32, kind="ExternalInput")
    i65 = nc.dram_tensor("i65", [DQ + 1, DQ + 1], F16, kind="ExternalInput")
    maskt = nc.dram_tensor("maskt", [N, NLOC], FP8, kind="ExternalInput")
    out = nc.dram_tensor("out", [NLOC, DQ], F32, kind="ExternalOutput")

    pers = ctx.enter_context(tc.tile_pool(name="pers", bufs=1))
    pm = ctx.enter_context(tc.tile_pool(name="pm", bufs=6))
    pw = ctx.enter_context(tc.tile_pool(name="pw", bufs=3))
    pfin = ctx.enter_context(tc.tile_pool(name="pfin", bufs=2))
    ps = ctx.enter_context(tc.tile_pool(name="ps", bufs=3, space="PSUM"))
    pacc = ctx.enter_context(tc.tile_pool(name="pacc", bufs=1, space="PSUM"))

    # ---- persistent SBUF ----
    xt_sb = [pers.tile([128, N], BF16, tag=f"xt{c}", name=f"xt{c}") for c in range(2)]
    xtq_sb = [pers.tile([128, NLOC], BF16, tag=f"xtq{c}", name=f"xtq{c}") for c in range(2)]
    wq_sb = [pers.tile([128, DQ], BF16, tag=f"wq{c}", name=f"wq{c}")
             for c in range(2)]
    wkv_sb = [pers.tile([128, 128], BF16, tag=f"wkv{c}", name=f"wkv{c}")
              for c in range(2)]
    for c in range(2):
        nc.sync.dma_start(wq_sb[c][:], wq[c * 128:(c + 1) * 128, :])
        nc.sync.dma_start(wkv_sb[c][:], wkv[c * 128:(c + 1) * 128, :])
    bq_sb = pers.tile([DQ, 1], F32, tag="bq")
    bkv_sb = pers.tile([128, 1], F32, tag="bkv")
    i65_sb = pers.tile([DQ + 1, DQ + 1], F16, tag="i65")
    nc.sync.dma_start(bq_sb[:], bq[:])
    nc.sync.dma_start(bkv_sb[:], bkv[:])
    nc.sync.dma_start(i65_sb[:], i65[:])
    qth_sb = pers.tile([DQ, NLOC], F16, tag="qth")
    kvt_sb = pers.tile([128, N], F16, tag="kvt")
    vh_sb = pers.tile([128, JT * VSLOT], F16, tag="vh")
    accT_sb = pers.tile([DQ + 1, NLOC], F16, tag="accT")
    o_sb = pers.tile([128, (NLOC // 128) * DQ], F32, tag="osb")

    # x^T streamed in segments, seg-major so both halves of segment 0 land
    # first and the projections can start early. The first few mask tiles
    # are prefetched between segment 0 and the bulk so the main loop's
    # head is not starved behind the full x stream.
    def xt_seg(s):
        for c in range(2):
            nc.sync.dma_start(
                xt_sb[c][:, s * XSEG:(s + 1) * XSEG],
                xt[c * 128:(c + 1) * 128, s * XSEG:(s + 1) * XSEG],
            )

    for c in range(2):
        nc.sync.dma_start(xtq_sb[c][:], xtq[c * 128:(c + 1) * 128, :])
    xt_seg(0)

    vh3 = vh_sb[:].rearrange("p (b e) -> p b e", e=VSLOT)
    nc.vector.memset(vh3[:, :, DQ:DQ + 1], 1.0)

    # ---- Q projection: q^T [64, 1024] fp16 (scale/bias folded on host) ----
    qt = ps.tile([128, NLOC], F32, tag="s", name="qt")
    for h in range(2):
        hs = slice(h * SEG, (h + 1) * SEG)
        nc.tensor.matmul(qt[:DQ, hs], wq_sb[0][:], xtq_sb[0][:, hs],
                         start=True, stop=False)
        nc.tensor.matmul(qt[:DQ, hs], wq_sb[1][:], xtq_sb[1][:, hs],
                         start=False, stop=True)
    nc.vector.tensor_scalar_add(qth_sb[:], qt[:DQ, :], bq_sb[:])

    # ---- joint K/V projection: kv^T [128, 8192] fp16; v tiles row-major
    # via batched DMA-XBAR transposes (16 tiles per call). Chunks 0-3 run
    # in the preamble; 4-15 interleave into the main loop so the PE stays
    # continuously busy (and in its fast p-state) instead of stalling on
    # the DVE drain round-trip. ----
    def kv_chunk(c16):
        cs = slice(c16 * SEG, (c16 + 1) * SEG)
        t = ps.tile([128, NLOC], F32, tag="s", name=f"kv{c16}")
        nc.tensor.matmul(t[:, 0:SEG], wkv_sb[0][:], xt_sb[0][:, cs],
                         start=True, stop=False)
        nc.tensor.matmul(t[:, 0:SEG], wkv_sb[1][:], xt_sb[1][:, cs],
                         start=False, stop=True)
        nc.vector.tensor_scalar_add(kvt_sb[:, cs], t[:, 0:SEG], bkv_sb[:])

    def v_transpose(c16_lo, c16_hi):
        jt0, jt1 = c16_lo * (SEG // 128), c16_hi * (SEG // 128)
        nc.sync.dma_start_transpose(
            vh3[:, jt0:jt1, 0:DQ],
            kvt_sb[DQ:128, c16_lo * SEG:c16_hi * SEG],
        )

    mask_tiles = {}
    def mask_fetch(jt):
        m_t = pm.tile([128, NLOC], FP8, tag="m", name=f"m{jt}")
        nc.sync.dma_start(m_t[:], maskt[jt * 128:(jt + 1) * 128, :])
        mask_tiles[jt] = m_t

    for jt in range(6):
        mask_fetch(jt)
    for s in range(1, N // XSEG):
        xt_seg(s)
    for c16 in range(4):
        kv_chunk(c16)
    v_transpose(0, 4)

    # ---- main loop over 64 column tiles ----
    acc = pacc.tile([DQ + 1, NLOC], F32, tag="acc")
    for jt in range(JT):
        if jt not in mask_tiles:
            mask_fetch(jt)
        m_t = mask_tiles.pop(jt)
        s_t = ps.tile([128, NLOC], F32, tag="s")
        kh = kvt_sb[0:DQ, jt * 128:(jt + 1) * 128]
        for h in range(2):
            hs = slice(h * SEG, (h + 1) * SEG)
            nc.tensor.matmul(s_t[:, hs], kh, qth_sb[:, hs],
                             start=True, stop=True)
        nc.vector.tensor_tensor(s_t[:], s_t[:], m_t[:], AO.add)
        w_t = pw.tile([128, NLOC], F16, tag="w")
        nc.scalar.activation(w_t[:], s_t[:], AF.Exp)
        vhb = vh3[:, jt, 0:DQ + 1]
        for h in range(2):
            hs = slice(h * SEG, (h + 1) * SEG)
            nc.tensor.matmul(acc[:, hs], vhb, w_t[:, hs],
                             start=(jt == 0), stop=(jt == JT - 1))
        if jt < 12:
            kv_chunk(4 + jt)
            if jt % 4 == 3:
                v_transpose(jt + 1, jt + 5)

    # ---- finish: transpose via matmul with I65, divide by Z, one DMA ----
    nc.scalar.activation(accT_sb[:], acc[:], AF.Copy)
    o3 = o_sb[:].rearrange("p (b e) -> p b e", e=DQ)
    for it in range(NLOC // 128):
        po = ps.tile([128, NLOC], F32, tag="s", name=f"po{it}")
        nc.tensor.matmul(po[:, 0:DQ + 1], accT_sb[:, it * 128:(it + 1) * 128],
                         i65_sb[:], start=True, stop=True)
        rz = pfin.tile([128, 1], F32, tag="rz")
        nc.vector.reciprocal(rz[:], po[:, DQ:DQ + 1])
        nc.vector.tensor_scalar_mul(o3[:, it, :], po[:, 0:DQ], rz[:])
    out3 = out[:].rearrange("(b p) e -> p b e", p=128)
    nc.scalar.dma_start(out3, o3[:, :, :])


_CACHE = {}


def _program():
    if "nc" not in _CACHE:
        import contextlib
        nc = bacc.Bacc("TRN2", target_bir_lowering=False, debug=False,
                       num_devices=NCORES)
        with tile.TileContext(nc) as tc:
            with contextlib.ExitStack() as ctx:
                _emit(nc, tc, ctx)
        nc.compile()
        _CACHE["nc"] = nc
    return _CACHE["nc"]


def kernel(**inputs):
    x = np.asarray(inputs["x"], dtype=np.float32)
    ei = np.asarray(inputs["edge_index"])
    Wq = np.asarray(inputs["Wq"], dtype=np.float32)
    bq = np.asarray(inputs["bq"], dtype=np.float32)
    Wk = np.asarray(inputs["Wk"], dtype=np.float32)
    bk = np.asarray(inputs["bk"], dtype=np.float32)
    Wv = np.asarray(inputs["Wv"], dtype=np.float32)
    bv = np.asarray(inputs["bv"], dtype=np.float32)

    scale = 1.0 / np.sqrt(np.float32(DQ))
    bf16 = ml_dtypes.bfloat16
    f8 = ml_dtypes.float8_e4m3
    xT = np.ascontiguousarray(x.T).astype(bf16)         # (256, 8192)
    wq_s = np.ascontiguousarray(Wq * scale).astype(bf16)
    wkv = np.ascontiguousarray(np.concatenate([Wk, Wv], axis=1)).astype(bf16)
    bq_s = np.ascontiguousarray((bq * scale).reshape(DQ, 1))
    bkv = np.ascontiguousarray(np.concatenate([bk, bv]).reshape(128, 1))
    i65 = np.eye(DQ + 1, dtype=np.float16)
    adj = np.zeros((N, N), dtype=np.bool_)
    adj[ei[0], ei[1]] = True

    in_maps = []
    for c in range(NCORES):
        rows = slice(c * NLOC, (c + 1) * NLOC)
        mt = np.where(adj[rows].T, np.float32(0.0), np.float32(MNEG))
        in_maps.append({
            "xt": xT,
            "xtq": np.ascontiguousarray(xT[:, rows]),
            "wq": wq_s, "wkv": wkv,
            "bq": bq_s, "bkv": bkv, "i65": i65,
            "maskt": np.ascontiguousarray(mt).astype(f8),
        })

    global _last_in_maps
    _last_in_maps = in_maps
    nc = _program()
    res = run_bass_kernel_spmd(nc, in_maps, core_ids=list(range(NCORES)))
    out = np.concatenate([res.results[c]["out"] for c in range(NCORES)], axis=0)
    return out.astype(np.float32)


_last_in_maps = None
